# revision 1
# baseline (speedup 1.0000x reference)
"""CBOW forward kernel for one TRN2 chip (8 NeuronCores), tensor-parallel on vocab.

Math (matches the reference):
    embed[b, c, :] = emb_W.T[contexts[b, c]] + emb_b          # gather
    out = embed.reshape(B, CTX*EMB) @ fc_W.T + fc_b           # [B, VOCAB]

Distribution: vocab dim sharded 8 ways (fc_W rows / fc_b / output columns);
contexts + emb table replicated so the gather is fully local — no collectives.
Each core computes out_shard.T = fc_W_shard @ embed.T  ->  [VSHARD, B], and the
host concatenates the shards and returns the transposed view.

Per-core device schedule:
  1. one DMA for indices, 4 indirect-DMA gathers of 256 B emb rows -> raw
     [128 batch, 512 feat] per batch tile
  2. PE transposes raw 128x128 blocks -> embT (K-major, bf16) -- the moving
     matmul operand
  3. main loop over 98 vocab tiles: 4 ldweights (fc bf16) + 16 matmuls
     (K=512 accumulated in PSUM) per tile, bias-add fused into the
     PSUM->SBUF drain (scalar/vector engines), 1 MiB contiguous output DMA

emb_b and fc_b are folded on the host into one effective bias
fc_be = fc_W @ tile(emb_b, CTX) + fc_b  (pure weight preprocessing).
"""

import os

import numpy as np
import ml_dtypes

import concourse.bacc as bacc
import concourse.bass as bass
import concourse.mybir as mybir
import concourse.tile as tile
from concourse.bass_utils import run_bass_kernel_spmd
from concourse.masks import make_identity

# Problem shape (hardcoded per harness contract).
VOCAB = 100000
CTX = 8
EMB = 64
BATCH = 2048
K = CTX * EMB            # 512 contraction dim
NCORES = 8
VSHARD = 12544           # 98 * 128, vocab cols per core (padded)
VPAD = VSHARD * NCORES   # 100352
NVT = VSHARD // 128      # 98 vocab tiles per core
VCHUNK = 7               # vocab tiles per fc_W DMA chunk
NCHUNK = NVT // VCHUNK   # 14
CHUNK_COLS = VCHUNK * 128  # 896
NBT = BATCH // 128       # 16 batch tiles
NBC = BATCH // 512       # 4 batch chunks (psum banks per vocab tile)

F32 = mybir.dt.float32
BF16 = mybir.dt.bfloat16
I32 = mybir.dt.int32
OUT_DT = BF16            # output quantization: rel err ~2e-3 << 2e-2 gate

_CACHE = {}


def _install_trace_hook():
    """Provide the missing antenv.axon_hooks module so trace=True works.

    The agent image's antenv lacks axon_hooks; recreate it and install the
    ctypes NTFF hook from trn_boot. Degrades silently on any failure.
    """
    import sys
    import types

    try:
        if "antenv.axon_hooks" not in sys.modules:
            mod = types.ModuleType("antenv.axon_hooks")
            mod._hook = None
            mod.set_axon_ntff_profile_hook = lambda h: setattr(mod, "_hook", h)
            mod.get_axon_ntff_profile_hook = lambda: mod._hook
            sys.modules["antenv.axon_hooks"] = mod
            import antenv

            antenv.axon_hooks = mod
        mod = sys.modules["antenv.axon_hooks"]
        if mod.get_axon_ntff_profile_hook() is None:
            if "/root/.axon_site/trn_agent_boot" not in sys.path:
                sys.path.insert(0, "/root/.axon_site/trn_agent_boot")
            import trn_boot

            mod.set_axon_ntff_profile_hook(
                trn_boot._ntff_profile_via_ctypes("/opt/axon/libaxon_pjrt.so")
            )
        return True
    except Exception as e:  # pragma: no cover
        print(f"trace hook install failed: {type(e).__name__}: {e}")
        return False


NJL = BATCH * CTX // NCORES // 128   # 16 gather calls per core


def _build_nc(probe_1core=False):
    nc = bacc.Bacc(
        "TRN2", target_bir_lowering=False, debug=False, num_devices=NCORES
    )
    idx_my = nc.declare_dram_parameter("idx_my", [128, NJL], I32, isOutput=False)
    emb_wt = nc.declare_dram_parameter("emb_wt", [VOCAB, EMB], F32, isOutput=False)
    fc_w = nc.declare_dram_parameter(
        "fc_w", [NCHUNK, 128, 4, CHUNK_COLS], BF16, isOutput=False
    )
    fc_be = nc.declare_dram_parameter("fc_be", [128, NVT], F32, isOutput=False)
    out = nc.declare_dram_parameter("out", [VSHARD, BATCH], OUT_DT, isOutput=True)

    with tile.TileContext(nc) as tc:
        with tc.tile_pool(name="const", bufs=1) as const:
            idx_sb = const.tile([128, NJL], I32, tag="idx", name="idx_sb")
            nc.sync.dma_start(out=idx_sb[:], in_=idx_my[:])
            ident = const.tile([128, 128], F32, tag="ident", name="ident")
            make_identity(nc, ident[:])
            fcbe_sb = const.tile([128, NVT], F32, tag="fcbe", name="fcbe_sb")
            nc.sync.dma_start(out=fcbe_sb[:], in_=fc_be[:])
            # warm the ACT Identity table before the main loop needs it
            actwarm = const.tile([128, 1], F32, tag="actwarm", name="actwarm")
            nc.scalar.activation(
                out=actwarm[:],
                in_=fcbe_sb[:, 0:1],
                func=mybir.ActivationFunctionType.Identity,
                bias=fcbe_sb[:, 0:1],
            )

            # Local gather of this core's 1/8 of the batch (one emb row per
            # partition per call): raw_loc[p, (ml*8+c)*64+e] for local batch
            # tiles ml in {0,1} (global m = 2s+ml).
            raw_loc = const.tile([128, NJL * EMB], F32, tag="rawloc", name="raw_loc")
            for jl in range(NJL):
                nc.gpsimd.indirect_dma_start(
                    out=raw_loc[:, jl * EMB : (jl + 1) * EMB],
                    out_offset=None,
                    in_=emb_wt[:],
                    in_offset=bass.IndirectOffsetOnAxis(
                        ap=idx_sb[:, jl : jl + 1], axis=0
                    ),
                )

            # Transpose the local slice to K-major bf16 BEFORE the exchange,
            # then AllGather the transposed slices: embT[k][q, b] =
            # embed_flat[b, k*128+q] with b = s*256 + (local col).
            LB = 128 * 2  # local batch count
            embT = []
            for k in range(4):
                t = const.tile([128, BATCH], BF16, tag=f"embT{k}", name=f"embT{k}")
                embT.append(t)
            embT_loc = const.tile([128, 4 * LB], BF16, tag="embTloc", name="embT_loc")
            with tc.tile_pool(name="tpsum", bufs=4, space="PSUM") as tps:
                for ml in range(2):
                    for k in range(4):
                        ps = tps.tile([128, 128], F32, tag="tps", name="tps")
                        nc.tensor.transpose(
                            ps[:],
                            raw_loc[:, ml * K + k * 128 : ml * K + (k + 1) * 128],
                            ident[:],
                        )
                        nc.vector.tensor_copy(
                            out=embT_loc[
                                :, k * LB + ml * 128 : k * LB + (ml + 1) * 128
                            ],
                            in_=ps[:],
                        )
            with tc.tile_pool(name="dramp", bufs=1, space="DRAM") as dramp:
                ag_in = dramp.tile([128, 4 * LB], BF16, tag="agin", name="ag_in")
                ag_out = dramp.tile(
                    [NCORES, 128, 4 * LB], BF16, tag="agout", name="ag_out",
                    addr_space="Shared",
                )
                nc.gpsimd.dma_start(out=ag_in[:], in_=embT_loc[:])
                if probe_1core:
                    # timing probe only: skip the collective, fill embT with
                    # the local slice (values wrong, timing representative)
                    for k in range(4):
                        for rep in range(NCORES):
                            nc.sync.dma_start(
                                out=embT[k][:, rep * LB : (rep + 1) * LB],
                                in_=ag_in[:, k * LB : (k + 1) * LB],
                            )
                else:
                    nc.gpsimd.collective_compute(
                        "AllGather",
                        mybir.AluOpType.bypass,
                        replica_groups=[list(range(NCORES))],
                        ins=[ag_in[:]],
                        outs=[ag_out[:]],
                    )
                    for k in range(4):
                        nc.sync.dma_start(
                            out=embT[k][:],
                            in_=ag_out[:, :, k * LB : (k + 1) * LB].rearrange(
                                "s p c -> p s c"
                            ),
                        )

            # Main loop: out.T[v*128:(v+1)*128, :] for 98 vocab tiles.
            with (
                tc.tile_pool(name="fcp", bufs=2) as fcp,
                tc.tile_pool(name="outp", bufs=4) as outp,
                tc.tile_pool(name="mpsum", bufs=2, space="PSUM") as mps,
            ):
                for ci in range(NCHUNK):
                    fct = fcp.tile([128, 4, CHUNK_COLS], BF16, tag="fct", name="fct")
                    nc.sync.dma_start(out=fct[:], in_=fc_w[ci])
                    for vt in range(VCHUNK):
                        v = ci * VCHUNK + vt
                        pss = [
                            mps.tile([128, 512], F32, tag=f"ps{bc}", name=f"ps{bc}")
                            for bc in range(NBC)
                        ]
                        for k in range(4):
                            lhsT = fct[:, k, vt * 128 : (vt + 1) * 128]
                            for bc in range(NBC):
                                nc.tensor.matmul(
                                    out=pss[bc][:],
                                    lhsT=lhsT,
                                    rhs=embT[k][:, bc * 512 : (bc + 1) * 512],
                                    start=(k == 0),
                                    stop=(k == 3),
                                )
                        osb = outp.tile([128, BATCH], OUT_DT, tag="osb", name="osb")
                        for bc in range(NBC):
                            if bc < 2:
                                nc.scalar.activation(
                                    out=osb[:, bc * 512 : (bc + 1) * 512],
                                    in_=pss[bc][:],
                                    func=mybir.ActivationFunctionType.Identity,
                                    bias=fcbe_sb[:, v : v + 1],
                                )
                            else:
                                nc.vector.tensor_scalar_add(
                                    out=osb[:, bc * 512 : (bc + 1) * 512],
                                    in0=pss[bc][:],
                                    scalar1=fcbe_sb[:, v : v + 1],
                                )
                        nc.sync.dma_start(
                            out=out[v * 128 : (v + 1) * 128, :], in_=osb[:]
                        )
    nc.compile()
    return nc


def _prep_inputs(contexts, emb_W, emb_b, fc_W, fc_b):
    contexts = np.asarray(contexts)
    emb_W = np.asarray(emb_W, dtype=np.float32)
    emb_b = np.asarray(emb_b, dtype=np.float32)
    fc_W = np.asarray(fc_W, dtype=np.float32)
    fc_b = np.asarray(fc_b, dtype=np.float32)

    # idx2d[j, p] = contexts[(j//8)*128 + p, j%8] with j = m*8+c; core s
    # gathers columns j in [16s, 16(s+1)) for its 1/8 of the batch.
    idx2d = (
        contexts.astype(np.int64).reshape(NBT, 128, CTX).transpose(0, 2, 1)
        .reshape(NBT * CTX, 128)
    )
    emb_wt = np.ascontiguousarray(emb_W.T)  # [VOCAB, 64] f32

    # effective bias: fc_be = fc_W @ tile(emb_b, CTX) + fc_b  (padded)
    emb_b_t = np.tile(emb_b, CTX)
    fc_be_full = (
        fc_W.astype(np.float64) @ emb_b_t.astype(np.float64)
        + fc_b.astype(np.float64)
    ).astype(np.float32)
    fc_be_pad = np.zeros(VPAD, dtype=np.float32)
    fc_be_pad[:VOCAB] = fc_be_full

    # fc_W.T padded to VPAD cols, bf16, chunked per-core layout
    fcT = np.zeros((K, VPAD), dtype=np.float32)
    fcT[:, :VOCAB] = fc_W.T
    fcT = fcT.astype(ml_dtypes.bfloat16)

    in_maps = []
    for s in range(NCORES):
        shard = fcT[:, s * VSHARD : (s + 1) * VSHARD]
        fc_host = np.ascontiguousarray(
            shard.reshape(4, 128, NCHUNK, CHUNK_COLS).transpose(2, 1, 0, 3)
        )
        be = np.ascontiguousarray(
            fc_be_pad[s * VSHARD : (s + 1) * VSHARD].reshape(NVT, 128).T
        )
        idx_my = np.ascontiguousarray(
            idx2d[s * NJL : (s + 1) * NJL, :].T.astype(np.int32)
        )
        in_maps.append(
            {"idx_my": idx_my, "emb_wt": emb_wt, "fc_w": fc_host, "fc_be": be}
        )
    return in_maps


def kernel(contexts, emb_W, emb_b, fc_W, fc_b):
    if "nc" not in _CACHE:
        _CACHE["nc"] = _build_nc()
    nc = _CACHE["nc"]
    in_maps = _prep_inputs(contexts, emb_W, emb_b, fc_W, fc_b)
    trace = bool(int(os.environ.get("KERNEL_TRACE", "0")))
    if trace:
        trace = _install_trace_hook()
    res = run_bass_kernel_spmd(
        nc, in_maps, core_ids=list(range(NCORES)), trace=trace
    )
    _CACHE["last_exec_time_ns"] = res.exec_time_ns
    full = np.concatenate(
        [np.asarray(r["out"]).astype(np.float32) for r in res.results], axis=0
    )
    return full[:VOCAB].T



# revision 3
# speedup vs baseline: 1.7330x; 1.7330x over previous
"""CBOW forward kernel for one TRN2 chip (8 NeuronCores), tensor-parallel on vocab.

Math (matches the reference):
    embed[b, c, :] = emb_W.T[contexts[b, c]] + emb_b          # gather
    out = embed.reshape(B, CTX*EMB) @ fc_W.T + fc_b           # [B, VOCAB]

Distribution: vocab dim sharded 8 ways (fc_W rows / fc_b / output columns);
contexts + emb table replicated; each core gathers 1/8 of the batch locally,
the transposed activation slices are exchanged with one AllGather.

Numerics: the GEMM runs in fp8 e4m3 with DoubleRow perf mode (2 fp8 K-planes
per PE pass -> 2x bf16 throughput). Both operands are quantized on the HOST
(x512 scale so sigma~10 sits in e4m3's normal range); activations travel
through gather/transpose as e4m3-grid values stored in bf16, so every device
conversion is exact. PSUM accumulates f32; the drain fuses x 2^-18 rescale +
per-partition bias (fc_be = fc_W @ tile(emb_b) + fc_b, f32) and emits bf16.
End-to-end rel err vs the f32 reference: 1.4e-2 (gate 2e-2).

Per-core device schedule:
  1. dummy 1-byte AllGather issued first to eat the CC firmware cold-start
  2. idx DMA + 16 indirect gathers (bf16 rows) + 8 PE transposes -> local
     K-major fp8 slice; one AllGather of 128KB exchanges the 8 slices
  3. all of fc (6.4MB fp8) is DMA'd into SBUF during step 2 (it stays
     resident; the main loop does no input DMA)
  4. main loop over 98 vocab tiles: 8 DoubleRow matmuls (K=512 as 2 passes
     of 2x128) into 4 PSUM banks, ACT/DVE drain with fused scale+bias,
     512KB contiguous output DMA per tile
"""

import os

import numpy as np
import ml_dtypes

import concourse.bacc as bacc
import concourse.bass as bass
import concourse.mybir as mybir
import concourse.tile as tile
from concourse.bass_utils import run_bass_kernel_spmd
from concourse.masks import make_identity

# Problem shape (hardcoded per harness contract).
VOCAB = 100000
CTX = 8
EMB = 64
BATCH = 2048
K = CTX * EMB            # 512 contraction dim
NCORES = 8
VSHARD = 12544           # 98 * 128, vocab cols per core (padded)
VPAD = VSHARD * NCORES   # 100352
NVT = VSHARD // 128      # 98 vocab tiles per core
NBC = BATCH // 512       # 4 batch chunks (psum banks per vocab tile)
NPASS = 2                # DoubleRow passes over K (2 x 256)
NI = 2                   # fp8 K-planes per pass
NJL = BATCH * CTX // NCORES // 128   # 16 gather calls per core
LB = 256                 # local batch rows per core

F32 = mybir.dt.float32
BF16 = mybir.dt.bfloat16
F8 = mybir.dt.float8e4
I32 = mybir.dt.int32
OUT_DT = BF16
E4NP = ml_dtypes.float8_e4m3fn

QSCALE = 512.0                       # per-operand fp8 scale
DRAIN_SCALE = 1.0 / (QSCALE * QSCALE)

_CACHE = {}


def _install_trace_hook():
    """Provide the missing antenv.axon_hooks module so trace=True works."""
    import sys
    import types

    try:
        if "antenv.axon_hooks" not in sys.modules:
            mod = types.ModuleType("antenv.axon_hooks")
            mod._hook = None
            mod.set_axon_ntff_profile_hook = lambda h: setattr(mod, "_hook", h)
            mod.get_axon_ntff_profile_hook = lambda: mod._hook
            sys.modules["antenv.axon_hooks"] = mod
            import antenv

            antenv.axon_hooks = mod
        mod = sys.modules["antenv.axon_hooks"]
        if mod.get_axon_ntff_profile_hook() is None:
            if "/root/.axon_site/trn_agent_boot" not in sys.path:
                sys.path.insert(0, "/root/.axon_site/trn_agent_boot")
            import trn_boot

            mod.set_axon_ntff_profile_hook(
                trn_boot._ntff_profile_via_ctypes("/opt/axon/libaxon_pjrt.so")
            )
        return True
    except Exception as e:  # pragma: no cover
        print(f"trace hook install failed: {type(e).__name__}: {e}")
        return False


def _build_nc(hostgather=False):
    nc = bacc.Bacc(
        "TRN2", target_bir_lowering=False, debug=False, num_devices=NCORES
    )
    if not hostgather:
        idx_my = nc.declare_dram_parameter("idx_my", [128, NJL], I32, isOutput=False)
        emb_wt = nc.declare_dram_parameter(
            "emb_wt", [VOCAB, EMB], BF16, isOutput=False
        )
    else:
        embt_h = nc.declare_dram_parameter(
            "embt_h", [128, NPASS * NI * 8 * LB], F8, isOutput=False
        )
    fc_w = nc.declare_dram_parameter(
        "fc_w", [128, NPASS * NVT * NI * 128], F8, isOutput=False
    )
    fc_be = nc.declare_dram_parameter("fc_be", [128, NVT], F32, isOutput=False)
    out = nc.declare_dram_parameter("out", [VSHARD, BATCH], OUT_DT, isOutput=True)

    with tile.TileContext(nc) as tc:
        with tc.tile_pool(name="const", bufs=1) as const:
            # embT2[pass][p, i, s, lb]: k = pass*256 + i*128 + p, b = s*256+lb
            embT2 = [
                const.tile([128, NI, 8, LB], F8, tag=f"embT2{p}", name=f"embT2{p}")
                for p in range(NPASS)
            ]
            # whole fc shard lives in SBUF: [p, pass, v, i, m]
            fc_sb = const.tile(
                [128, NPASS, NVT, NI, 128], F8, tag="fcsb", name="fc_sb"
            )
            fcbe_sb = const.tile([128, NVT], F32, tag="fcbe", name="fcbe_sb")

            if not hostgather:
                with tc.tile_pool(name="dramp", bufs=1, space="DRAM") as dramp:
                    dum_in = dramp.tile([1, 1], mybir.dt.uint8, tag="dumin",
                                        name="dum_in")
                    dum_out = dramp.tile([NCORES, 1], mybir.dt.uint8, tag="dumout",
                                         name="dum_out", addr_space="Shared")
                    # warm the CC firmware before the real AllGather needs it
                    nc.gpsimd.collective_compute(
                        "AllGather",
                        mybir.AluOpType.bypass,
                        replica_groups=[list(range(NCORES))],
                        ins=[dum_in[:]],
                        outs=[dum_out[:]],
                    )

                    idx_sb = const.tile([128, NJL], I32, tag="idx", name="idx_sb")
                    nc.sync.dma_start(out=idx_sb[:], in_=idx_my[:])
                    ident = const.tile([128, 128], BF16, tag="ident", name="ident")
                    make_identity(nc, ident[:])

                    # local gather: one emb row (bf16, e4m3-grid values) per
                    # partition per call; raw col (ml*8+c)*64+e
                    raw_loc = const.tile([128, NJL * EMB], BF16, tag="rawloc",
                                         name="raw_loc")
                    for jl in range(NJL):
                        nc.gpsimd.indirect_dma_start(
                            out=raw_loc[:, jl * EMB : (jl + 1) * EMB],
                            out_offset=None,
                            in_=emb_wt[:],
                            in_offset=bass.IndirectOffsetOnAxis(
                                ap=idx_sb[:, jl : jl + 1], axis=0
                            ),
                        )

                    # transpose local slice to K-major, convert (exactly) to fp8
                    embT_loc = const.tile([128, NPASS, NI, LB], F8, tag="embTloc",
                                          name="embT_loc")
                    with tc.tile_pool(name="tpsum", bufs=4, space="PSUM") as tps:
                        for ml in range(2):
                            for t in range(4):
                                ps = tps.tile([128, 128], BF16, tag="tps", name="tps")
                                nc.tensor.transpose(
                                    ps[:],
                                    raw_loc[:, ml * K + t * 128 : ml * K + (t + 1) * 128],
                                    ident[:],
                                )
                                nc.vector.tensor_copy(
                                    out=embT_loc[
                                        :, t // 2, t % 2, ml * 128 : (ml + 1) * 128
                                    ],
                                    in_=ps[:],
                                )

                    ag_in = dramp.tile([128, NPASS * NI * LB], F8, tag="agin",
                                       name="ag_in")
                    ag_out = dramp.tile(
                        [NCORES, 128, NPASS * NI * LB], F8, tag="agout",
                        name="ag_out", addr_space="Shared",
                    )
                    nc.gpsimd.dma_start(out=ag_in[:], in_=embT_loc[:])
                    nc.gpsimd.collective_compute(
                        "AllGather",
                        mybir.AluOpType.bypass,
                        replica_groups=[list(range(NCORES))],
                        ins=[ag_in[:]],
                        outs=[ag_out[:]],
                    )
                    ag5 = ag_out[:].rearrange(
                        "s p (pp i c) -> s p pp i c", pp=NPASS, i=NI
                    )
                    for p in range(NPASS):
                        for i in range(NI):
                            nc.sync.dma_start(
                                out=embT2[p][:, i, :, :],
                                in_=ag5[:, :, p, i, :].rearrange("s p c -> p s c"),
                            )
            else:
                eh = embt_h[:].rearrange(
                    "p (pp i s c) -> p pp i s c", pp=NPASS, i=NI, s=8
                )
                for p in range(NPASS):
                    nc.sync.dma_start(out=embT2[p][:], in_=eh[:, p, :, :, :])

            # fc + bias loads (overlap the gather/collective)
            fcv = fc_w[:].rearrange(
                "p (pp v i m) -> p pp v i m", pp=NPASS, v=NVT, i=NI
            )
            for h in range(4):
                vs = (NVT // 4 + 1) if h < NVT % 4 else NVT // 4
                v0 = min(h, NVT % 4) * (NVT // 4 + 1) + max(0, h - NVT % 4) * (NVT // 4)
                nc.sync.dma_start(
                    out=fc_sb[:, :, v0 : v0 + vs, :, :], in_=fcv[:, :, v0 : v0 + vs, :, :]
                )
            nc.sync.dma_start(out=fcbe_sb[:], in_=fc_be[:])
            # warm the ACT Identity table before the main loop needs it
            actwarm = const.tile([128, 1], F32, tag="actwarm", name="actwarm")
            nc.scalar.activation(
                out=actwarm[:],
                in_=fcbe_sb[:, 0:1],
                func=mybir.ActivationFunctionType.Identity,
                bias=fcbe_sb[:, 0:1],
            )

            # Main loop: out.T[v*128:(v+1)*128, :] for 98 vocab tiles.
            with (
                tc.tile_pool(name="outp", bufs=4) as outp,
                tc.tile_pool(name="mpsum", bufs=2, space="PSUM") as mps,
            ):
                for v in range(NVT):
                    pss = [
                        mps.tile([128, 512], F32, tag=f"ps{bc}", name=f"ps{bc}")
                        for bc in range(NBC)
                    ]
                    for pp in range(NPASS):
                        lhsT = fc_sb[:, pp, v, :, :]
                        for bc in range(NBC):
                            nc.tensor.matmul(
                                out=pss[bc][:],
                                lhsT=lhsT,
                                rhs=embT2[pp][:, :, 2 * bc : 2 * bc + 2, :],
                                start=(pp == 0),
                                stop=(pp == NPASS - 1),
                                perf_mode=mybir.MatmulPerfMode.DoubleRow,
                            )
                    osb = outp.tile([128, BATCH], OUT_DT, tag="osb", name="osb")
                    for bc in range(NBC):
                        if bc < 2:
                            nc.scalar.activation(
                                out=osb[:, bc * 512 : (bc + 1) * 512],
                                in_=pss[bc][:],
                                func=mybir.ActivationFunctionType.Identity,
                                bias=fcbe_sb[:, v : v + 1],
                                scale=DRAIN_SCALE,
                            )
                        else:
                            nc.vector.tensor_scalar(
                                out=osb[:, bc * 512 : (bc + 1) * 512],
                                in0=pss[bc][:],
                                scalar1=DRAIN_SCALE,
                                scalar2=fcbe_sb[:, v : v + 1],
                                op0=mybir.AluOpType.mult,
                                op1=mybir.AluOpType.add,
                            )
                    nc.sync.dma_start(
                        out=out[v * 128 : (v + 1) * 128, :], in_=osb[:]
                    )
    nc.compile()
    return nc


def _prep_inputs(contexts, emb_W, emb_b, fc_W, fc_b, hostgather=False):
    contexts = np.asarray(contexts)
    emb_W = np.asarray(emb_W, dtype=np.float32)
    emb_b = np.asarray(emb_b, dtype=np.float32)
    fc_W = np.asarray(fc_W, dtype=np.float32)
    fc_b = np.asarray(fc_b, dtype=np.float32)
    NBT = BATCH // 128

    # effective bias: fc_be = fc_W @ tile(emb_b, CTX) + fc_b  (padded)
    emb_b_t = np.tile(emb_b, CTX)
    fc_be_full = (
        fc_W.astype(np.float64) @ emb_b_t.astype(np.float64)
        + fc_b.astype(np.float64)
    ).astype(np.float32)
    fc_be_pad = np.zeros(VPAD, dtype=np.float32)
    fc_be_pad[:VOCAB] = fc_be_full

    # fc_W.T padded, quantized e4m3 (x512), laid out [p, pass, v, i, m]
    fcT = np.zeros((K, VPAD), dtype=np.float32)
    fcT[:, :VOCAB] = fc_W.T
    fc8 = (fcT * QSCALE).astype(E4NP)
    fc8 = fc8.reshape(NPASS, NI, 128, NCORES, NVT, 128)

    in_maps = []
    if hostgather:
        embed = emb_W.T[contexts].reshape(BATCH, K)           # [B, K]
        e8 = (embed.T * QSCALE).astype(E4NP)                  # [K, B]
        e8 = np.ascontiguousarray(
            e8.reshape(NPASS, NI, 128, 8, LB).transpose(2, 0, 1, 3, 4)
        ).reshape(128, NPASS * NI * 8 * LB)
    else:
        # idx2d[j, p] = contexts[(j//8)*128 + p, j%8], j = m*8+c; core s gathers
        # columns j in [16s, 16(s+1))
        idx2d = (
            contexts.astype(np.int64).reshape(NBT, 128, CTX).transpose(0, 2, 1)
            .reshape(NBT * CTX, 128)
        )
        # emb table: e4m3-grid values (x512) stored exactly in bf16
        emb_wt = np.ascontiguousarray(
            (emb_W.T * QSCALE).astype(E4NP).astype(ml_dtypes.bfloat16)
        )

    for s in range(NCORES):
        fc_host = np.ascontiguousarray(
            fc8[:, :, :, s].transpose(2, 0, 3, 1, 4)
        ).reshape(128, NPASS * NVT * NI * 128)
        be = np.ascontiguousarray(
            fc_be_pad[s * VSHARD : (s + 1) * VSHARD].reshape(NVT, 128).T
        )
        m = {"fc_w": fc_host, "fc_be": be}
        if hostgather:
            m["embt_h"] = e8
        else:
            m["idx_my"] = np.ascontiguousarray(
                idx2d[s * NJL : (s + 1) * NJL, :].T.astype(np.int32)
            )
            m["emb_wt"] = emb_wt
        in_maps.append(m)
    return in_maps


def kernel(contexts, emb_W, emb_b, fc_W, fc_b):
    hostgather = bool(int(os.environ.get("KERNEL_HOSTGATHER", "0")))
    key = ("nc", hostgather)
    if key not in _CACHE:
        _CACHE[key] = _build_nc(hostgather=hostgather)
    nc = _CACHE[key]
    in_maps = _prep_inputs(contexts, emb_W, emb_b, fc_W, fc_b, hostgather)
    trace = bool(int(os.environ.get("KERNEL_TRACE", "0")))
    if trace:
        trace = _install_trace_hook()
    res = run_bass_kernel_spmd(
        nc, in_maps, core_ids=list(range(NCORES)), trace=trace
    )
    _CACHE["last_exec_time_ns"] = res.exec_time_ns
    full = np.concatenate(
        [np.asarray(r["out"]).astype(np.float32) for r in res.results], axis=0
    )
    return full[:VOCAB].T
